# revision 1
# baseline (speedup 1.0000x reference)
"""Trainium2 Bass kernel for the deterministic legality module.

Computes, for each board b, filter f and top-left placement (i,j):
    legal[b,f,i,j] = 1.0 iff every occupied cell of filter f, placed at
    (i,j), lands in-bounds on a free cell of board b (and f is non-empty).

Reformulated as one matmul per output tile:
    out[b, f*81+ij] = relu( sum_k boardX[b,k] * M[k, f*81+ij] )
where rows 0..80 of M hold filter f placed at ij (zero out of bounds) and
rows 81,82 hold the two integer halves of thr[f] = 1-area (or -1 for empty
filters); boardX appends two ones-columns to the board.  corr <= area
always, so relu(corr + thr) is exactly the 0/1 legality.

Both M and the transposed board are built on the HOST in numpy and
uploaded in fp8e4 (entries are 0/1 or integers in [-12,0] -- exact in
e4m3), padded to 128 partitions: the DMA engines have fixed partition
affinity, so 128-partition transfers fan out across all 16 engines while
narrow ones serialize.  The device is then a single pipeline:
  matmul (fp8, PSUM f32) -> relu+fp8 downcast (DVE/ACT, 5:6 split)
  -> HBM store (fp8, upcast to f32 on host).
All matmuls contract over the full 128 partitions (pad rows are zero, and
matmul time is N-bound) because the HAM clock gate keys on PE array row
occupancy: partial-K matmuls re-throttle the PE to 1.2 GHz even at 100%
busy, while full-K work holds it at 2.4 GHz.

Sharding: pure data parallelism, batch 4096 -> 512 per core on 8 cores.
"""

import numpy as np
import ml_dtypes

N_CORES = 8
BATCH = 4096
BPC = BATCH // N_CORES  # 512 boards per core
NPOS = 81               # 9x9 board cells / placements
NF = 264                # filters
NCOL = NF * NPOS        # 21384 output columns per board
K = NPOS + 2            # contraction: 81 board cells + 2 threshold rows
KPAD = 128              # uploads padded to 128 partitions for DMA fan-out

COL_TILE = 512          # one PSUM bank of f32
GRP = 1024              # 2 banks per PSUM ring slot / relu op
DMA_GRP = 4096          # output staging tile / store DMA (tail stays 904)
N_SLABS = 8             # M upload slabs after the 512-col prefetch slab
# The HAM clock gate keys on PE array occupancy: K=83 matmuls (65% of the
# 128 rows) read as "idle" and the clock re-throttles to 1.2 GHz even when
# the PE is 100% busy.  All matmuls therefore contract over the full 128
# partitions -- the pad rows are zero on both sides, and matmul time is
# N-bound, so K=128 is free, self-lifts the gate ~3.4us into the main
# loop, and holds it at 2.4 GHz (no separate warm-up needed).
# DVE:ACT relu split, retuned from measured op times (1209 vs 1083 ns)
DVE_SLOTS = (0, 2, 4, 6, 8, 10, 12, 14)
PERIOD = 17


def _build_m(filters: np.ndarray, areas: np.ndarray) -> np.ndarray:
    """M [128, 21384] fp8e4: placed-filter geometry + threshold rows + pad."""
    F = np.asarray(filters, dtype=np.float32).reshape(NF, 5, 5)
    M = np.zeros((KPAD, NF, NPOS), dtype=np.float32)
    for i in range(9):
        h = min(5, 9 - i)
        for j in range(9):
            w = min(5, 9 - j)
            blk = np.zeros((NF, 9, 9), dtype=np.float32)
            blk[:, i:i + h, j:j + w] = F[:, :h, :w]
            M[:NPOS, :, i * 9 + j] = blk.reshape(NF, NPOS).T
    ar = np.asarray(areas, dtype=np.float32).reshape(NF)
    thr = np.where(ar > 0.5, 1.0 - ar, -1.0)
    lo = np.floor(thr / 2.0)
    M[NPOS, :, :] = lo[:, None]          # floor(thr/2)   in [-12, 0]
    M[NPOS + 1, :, :] = (thr - lo)[:, None]  # ceil(thr/2) in [-12, 0]
    return M.reshape(KPAD, NCOL).astype(ml_dtypes.float8_e4m3)


def _build_boardt(board_free: np.ndarray) -> np.ndarray:
    """boardT [cores, 128, 512] fp8e4: transposed boards + ones rows + pad."""
    b = np.asarray(board_free, dtype=np.float32).reshape(N_CORES, BPC, NPOS)
    bt = np.zeros((N_CORES, KPAD, BPC), dtype=np.float32)
    bt[:, :NPOS, :] = b.transpose(0, 2, 1)
    bt[:, NPOS:K, :] = 1.0
    return bt.astype(ml_dtypes.float8_e4m3)


def _build_module():
    import concourse.bass as bass
    import concourse.mybir as mybir
    import concourse.tile as tile

    f32 = mybir.dt.float32
    fp8 = mybir.dt.float8e4
    Relu = mybir.ActivationFunctionType.Relu

    nc = bass.Bass("TRN2", target_bir_lowering=False, debug=False,
                   num_devices=N_CORES)

    boardt_d = nc.dram_tensor("boardt", [KPAD, BPC], fp8, kind="ExternalInput")
    m_d = nc.dram_tensor("mmat", [KPAD, NCOL], fp8, kind="ExternalInput")
    out_d = nc.dram_tensor("out", [BPC, NCOL], fp8, kind="ExternalOutput")

    with tile.TileContext(nc) as tc:
        with tc.tile_pool(name="const", bufs=1) as cpool:
            boardT = cpool.tile([KPAD, BPC], fp8)
            msb = cpool.tile([KPAD, NCOL], fp8)

            # M slabs on the SP hwdge ring (slab 0 configured first so the
            # first matmuls start as early as possible), boardT on the ACT
            # ring so both uploads run in parallel; output stores follow on
            # the SP ring, FIFO behind the slabs they never contend with.
            # geometric prefetch taper: tiny leading slabs so the first
            # matmuls never stall on a big in-flight upload
            bounds = [0, 512, 1536, 3584]
            step = -(-(NCOL - bounds[-1]) // 7)
            while bounds[-1] < NCOL:
                bounds.append(min(NCOL, bounds[-1] + step))
            for s0, s1 in zip(bounds[:-1], bounds[1:]):
                nc.sync.dma_start(msb[:, s0:s1], m_d[:, s0:s1])
            nc.scalar.dma_start(boardT[:], boardt_d[:])

            # ---- pipeline: PSUM ring (4 slots) + staging ---------------
            with (
                tc.tile_pool(name="wprep", bufs=1) as wprep,
                tc.tile_pool(name="psM", bufs=4, space="PSUM") as psM,
                tc.tile_pool(name="ostage", bufs=6) as ostage,
            ):
                # Preload the ACT activation table so the first real relu
                # doesn't pay the ~1.3us table load; source from a memset
                # scratch so it can run during the input upload.
                wrd = wprep.tile([32, 1], f32, tag="wrd")
                wz = wprep.tile([128, 256], f32, tag="wz")
                nc.vector.memset(wz[:], 0.0)
                nc.scalar.activation(wrd[0:1, 0:1], wz[0:1, 0:1], Relu)

                # compact full-K fp32 warm-up while inputs upload: lifts the
                # HAM clock gate so the first real matmuls already run at
                # 2.4 GHz (measured: ~9 cold matmuls otherwise).  Borrows a
                # PSUM ring slot; the ring reuses it afterwards.
                wps = psM.tile([128, GRP], f32, tag="mm")
                for _ in range(4):
                    nc.tensor.matmul(wps[:, 0:256], wz[:, 0:128], wz[:],
                                     start=True, stop=True)

                grp = 0
                nkb = BPC // 128
                for kb in range(nkb):
                    lhsT = boardT[:, kb * 128:(kb + 1) * 128]
                    # smaller stores on the final chunk: they release more
                    # often, so the DMA stream finishes with the drains
                    # instead of 3-4us after them
                    dgrp = DMA_GRP if kb < nkb - 1 else DMA_GRP // 2
                    for g0 in range(0, NCOL, dgrp):
                        dw = min(dgrp, NCOL - g0)
                        last = kb == nkb - 1 and g0 + dw >= NCOL
                        ot = ostage.tile([128, DMA_GRP], fp8, tag="ot")
                        for h0 in range(0, dw, GRP):
                            hw = min(GRP, dw - h0)
                            pt = psM.tile([128, GRP], f32, tag="mm")
                            for q in range(0, hw, COL_TILE):
                                w = min(COL_TILE, hw - q)
                                c = g0 + h0 + q
                                nc.tensor.matmul(pt[:, q:q + w], lhsT,
                                                 msb[:, c:c + w],
                                                 start=True, stop=True)
                            if last and h0 + hw >= dw:
                                # final group: drain on both engines so the
                                # closing store starts as early as possible
                                hh = hw // 2
                                nc.vector.tensor_scalar_max(
                                    ot[:, h0:h0 + hh], pt[:, :hh], 0.0)
                                nc.scalar.activation(ot[:, h0 + hh:h0 + hw],
                                                     pt[:, hh:hw], Relu)
                            elif grp % PERIOD in DVE_SLOTS and grp != 34:
                                nc.vector.tensor_scalar_max(
                                    ot[:, h0:h0 + hw], pt[:, :hw], 0.0)
                            else:
                                nc.scalar.activation(ot[:, h0:h0 + hw],
                                                     pt[:, :hw], Relu)
                            grp += 1
                        # the final store goes on the otherwise-idle ACT
                        # ring so it never queues behind the prior 4 KB
                        # transfer still draining on the SP ring
                        eng = nc.scalar if last else nc.sync
                        eng.dma_start(
                            out_d[kb * 128:(kb + 1) * 128, g0:g0 + dw],
                            ot[:, :dw])
    return nc


def _legalize_multiwait(nc):
    """Split multi-wait instructions for this walrus build.

    The TPB instruction encodings carry exactly one semaphore wait, and
    the walrus codegen here refuses instructions with more ("Too many
    sync wait commands").  Hoist all but one wait onto EventSemaphore
    carrier instructions placed immediately before, on the same engine —
    the sequencer blocks on each carrier first, which is semantically
    identical.
    """
    import concourse.mybir as mybir

    for func in nc.m.functions:
        for blk in func.blocks:
            out = []
            changed = False
            for inst in blk.instructions:
                si = inst.sync_info
                waits = list(si.on_wait) if si is not None and si.on_wait else []
                if len(waits) > 1:
                    for j, w in enumerate(waits[:-1]):
                        carrier = mybir.InstEventSemaphore(
                            name=f"{inst.name}-xw{j}",
                            engine=inst.engine,
                            ins=[], outs=[],
                            sync_info=mybir.SyncInfo(on_wait=[w],
                                                     on_update=[]),
                        )
                        nc.register_instruction(carrier)
                        out.append(carrier)
                    inst.sync_info = mybir.SyncInfo(
                        on_wait=[waits[-1]],
                        on_update=list(si.on_update) if si.on_update else [])
                    changed = True
                out.append(inst)
            if changed:
                blk.instructions = out


_MODULE = None


def _get_module():
    global _MODULE
    if _MODULE is None:
        _MODULE = _build_module()
        _legalize_multiwait(_MODULE)
    return _MODULE


def run(board_free, filters, areas, trace=False, **spmd_kwargs):
    from concourse.bass_utils import run_bass_kernel_spmd

    boardt = _build_boardt(board_free)
    mmat = _build_m(filters, areas)

    in_maps = [
        {"boardt": boardt[c], "mmat": mmat}
        for c in range(N_CORES)
    ]
    nc = _get_module()
    res = run_bass_kernel_spmd(nc, in_maps, core_ids=list(range(N_CORES)),
                               trace=trace, **spmd_kwargs)
    out = np.concatenate(
        [np.asarray(r["out"]).astype(np.float32) for r in res.results], axis=0)
    out = out.reshape(BATCH, NF, 9, 9)
    return out, res


def kernel(board_free, filters, areas):
    out, _ = run(board_free, filters, areas)
    return out



# revision 2
# speedup vs baseline: 1.9402x; 1.9402x over previous
"""Trainium2 Bass kernel for the deterministic legality module.

Computes, for each board b, filter f and top-left placement (i,j):
    legal[b,f,i,j] = 1.0 iff every occupied cell of filter f, placed at
    (i,j), lands in-bounds on a free cell of board b (and f is non-empty).

Reformulated as one matmul per output tile:
    out[b, c] = relu( sum_k boardX[b,k] * M[k, c] )
where rows 0..80 of M hold filter f placed at position p (zero out of
bounds) for column c=(f,p), and rows 81,82 hold the two integer halves
of thr[f] = 1-area (exact in fp8e4m3); boardX appends two ones-columns.
corr <= area always, so relu(corr + thr) is exactly the 0/1 legality.

Feasibility pruning: a filter whose max occupied row is r and max
occupied col is c can only be legal at the (9-r)*(9-c) top-left
positions where its footprint stays in bounds -- every other (f,p)
column is constant zero.  Only the ~32% feasible columns are computed
on device (the host scatters them back into the full output), cutting
PE columns, PSUM drain and HBM store traffic ~3x.  This is the main
speedup over the dense version.

Pipeline per core (batch 4096 -> 512 boards, 4 blocks of 128):
  matmul fp8 (K=128 full contraction, N<=512) -> PSUM ring (4 slots of
  1024 f32 cols) -> relu+fp8 drain split across DVE/ACT by a greedy
  time-balance on their measured cost models ((120+FD)/0.96GHz vs
  (172+FD)/1.2GHz; the PSUM read port is the pipeline bottleneck)
  -> SBUF staging -> HBM store on the SP HWDGE ring.
M uploads on the SP ring (tapered slabs), boardT on the ACT ring.
fp8 warmup matmuls on memset zeros keep the PE busy from the preamble
so the HAM clock gate (~4.5us qualification) lifts as early as
possible; full-K contraction keeps it lifted (pad rows are zero).
"""

import numpy as np
import ml_dtypes

N_CORES = 8
BATCH = 4096
BPC = BATCH // N_CORES  # 512 boards per core
NPOS = 81               # 9x9 board cells / placements
NF = 264                # filters
NCOL = NF * NPOS        # 21384 full output columns per board
K = NPOS + 2            # contraction: 81 board cells + 2 threshold rows
KPAD = 128              # uploads padded to 128 partitions for DMA fan-out

COL_TILE = 512          # one PSUM bank of f32
GRP = 1024              # PSUM ring slot / one drain op
DMA_GRP = 4096          # output staging tile / store DMA
# measured drain cost models (ns) for a PSUM->SBUF op of FD f32 columns
_DVE_NS = lambda fd: (120.0 + fd) / 0.96
_ACT_NS = lambda fd: (172.0 + fd) / 1.2


def _feasible_cols(filters: np.ndarray) -> np.ndarray:
    """Column indices (f*81 + i*9 + j) that can ever be legal."""
    F = np.asarray(filters, dtype=np.float32).reshape(NF, 5, 5) > 0.5
    cols = []
    for f in range(NF):
        occ = F[f]
        if not occ.any():
            continue
        rmax = int(np.where(occ.any(axis=1))[0].max())
        cmax = int(np.where(occ.any(axis=0))[0].max())
        for i in range(9 - rmax):
            for j in range(9 - cmax):
                cols.append(f * NPOS + i * 9 + j)
    return np.asarray(cols, dtype=np.int64)


def _build_m(filters: np.ndarray, areas: np.ndarray,
             cols: np.ndarray) -> np.ndarray:
    """M [128, ncolf] fp8e4: placed-filter geometry + threshold rows."""
    F = np.asarray(filters, dtype=np.float32).reshape(NF, 5, 5)
    M = np.zeros((KPAD, NF, NPOS), dtype=np.float32)
    for i in range(9):
        h = min(5, 9 - i)
        for j in range(9):
            w = min(5, 9 - j)
            blk = np.zeros((NF, 9, 9), dtype=np.float32)
            blk[:, i:i + h, j:j + w] = F[:, :h, :w]
            M[:NPOS, :, i * 9 + j] = blk.reshape(NF, NPOS).T
    ar = np.asarray(areas, dtype=np.float32).reshape(NF)
    thr = np.where(ar > 0.5, 1.0 - ar, -1.0)
    lo = np.floor(thr / 2.0)
    M[NPOS, :, :] = lo[:, None]              # floor(thr/2)  in [-12, 0]
    M[NPOS + 1, :, :] = (thr - lo)[:, None]  # ceil(thr/2)   in [-12, 0]
    return M.reshape(KPAD, NCOL)[:, cols].astype(ml_dtypes.float8_e4m3)


def _build_boardt(board_free: np.ndarray) -> np.ndarray:
    """boardT [cores, 128, 512] fp8e4: transposed boards + ones rows."""
    b = np.asarray(board_free, dtype=np.float32).reshape(N_CORES, BPC, NPOS)
    bt = np.zeros((N_CORES, KPAD, BPC), dtype=np.float32)
    bt[:, :NPOS, :] = b.transpose(0, 2, 1)
    bt[:, NPOS:K, :] = 1.0
    return bt.astype(ml_dtypes.float8_e4m3)


def _drain_plan(ncolf: int):
    """Greedy DVE/ACT time-balanced assignment of drain groups.

    Returns [(block, col0, fd, engine)] over all 4 blocks, in issue
    order; engine is 'v' (DVE) or 's' (ACT).
    """
    plan = []
    tv = ts = 0.0
    for kb in range(4):
        for g0 in range(0, ncolf, GRP):
            fd = min(GRP, ncolf - g0)
            last = kb == 3 and g0 + fd >= ncolf
            if last:
                plan.append((kb, g0, fd, 'split'))
                continue
            if tv + _DVE_NS(fd) <= ts + _ACT_NS(fd):
                tv += _DVE_NS(fd)
                plan.append((kb, g0, fd, 'v'))
            else:
                ts += _ACT_NS(fd)
                plan.append((kb, g0, fd, 's'))
    return plan


def _build_module(ncolf: int):
    import concourse.bass as bass
    import concourse.mybir as mybir
    import concourse.tile as tile

    f32 = mybir.dt.float32
    fp8 = mybir.dt.float8e4
    Relu = mybir.ActivationFunctionType.Relu

    nc = bass.Bass("TRN2", target_bir_lowering=False, debug=False,
                   num_devices=N_CORES)

    boardt_d = nc.dram_tensor("boardt", [KPAD, BPC], fp8, kind="ExternalInput")
    m_d = nc.dram_tensor("mmat", [KPAD, ncolf], fp8, kind="ExternalInput")
    out_d = nc.dram_tensor("out", [BPC, ncolf], fp8, kind="ExternalOutput")

    plan = _drain_plan(ncolf)

    with tile.TileContext(nc) as tc:
        with tc.tile_pool(name="const", bufs=1) as cpool:
            boardT = cpool.tile([KPAD, BPC], fp8)
            msb = cpool.tile([KPAD, ncolf], fp8)

            # M slabs on the SP hwdge ring with a geometric taper (small
            # leading slabs so the first matmuls start early); boardT on
            # the ACT ring so both uploads run in parallel.  Output
            # stores follow on the SP ring, FIFO behind the slabs.
            bounds = [0, 512, 1536, 3584]
            while bounds[-1] < ncolf:
                bounds.append(min(ncolf, bounds[-1] + 3200))
            for s0, s1 in zip(bounds[:-1], bounds[1:]):
                nc.sync.dma_start(msb[:, s0:s1], m_d[:, s0:s1])
            nc.scalar.dma_start(boardT[:], boardt_d[:])

            with (
                tc.tile_pool(name="wprep", bufs=1) as wprep,
                tc.tile_pool(name="psM", bufs=4, space="PSUM") as psM,
                tc.tile_pool(name="ostage", bufs=4) as ostage,
            ):
                # Preload the ACT activation table so the first real relu
                # doesn't pay the ~1.3us table load; source from a memset
                # scratch so it can run during the input upload.
                wrd = wprep.tile([32, 1], f32, tag="wrd")
                wz = wprep.tile([128, 512], fp8, tag="wz")
                nc.vector.memset(wz[:], 0.0)
                nc.scalar.activation(wrd[0:1, 0:1], wz[0:1, 0:1], Relu)

                # fp8 full-K warm-up matmuls while inputs upload: keeps
                # the PE busy from the preamble so the HAM clock gate
                # lifts as early as possible.  Borrows one PSUM ring
                # slot; the ring reuses it afterwards.
                wps = psM.tile([128, GRP], f32, tag="mm")
                for _ in range(6):
                    nc.tensor.matmul(wps[:, 0:512], wz[:, 0:128], wz[:],
                                     start=True, stop=True)

                nkb = BPC // 128
                cur_stage = {}
                for kb in range(nkb):
                    lhsT = boardT[:, kb * 128:(kb + 1) * 128]
                    # stage boundaries for this block's stores
                    dgrp = DMA_GRP if kb < nkb - 1 else 2048
                    stage_bounds = list(range(0, ncolf, dgrp)) + [ncolf]
                    si = 0
                    ot = None
                    for (pkb, g0, fd, eng) in plan:
                        if pkb != kb:
                            continue
                        if ot is None:
                            s0, s1 = stage_bounds[si], stage_bounds[si + 1]
                            ot = ostage.tile([128, DMA_GRP], fp8, tag="ot")
                        pt = psM.tile([128, GRP], f32, tag="mm")
                        for q in range(0, fd, COL_TILE):
                            w = min(COL_TILE, fd - q)
                            nc.tensor.matmul(pt[:, q:q + w], lhsT,
                                             msb[:, g0 + q:g0 + q + w],
                                             start=True, stop=True)
                        o0 = g0 - s0
                        if eng == 'v':
                            nc.vector.tensor_scalar_max(
                                ot[:, o0:o0 + fd], pt[:, :fd], 0.0)
                        elif eng == 's':
                            nc.scalar.activation(ot[:, o0:o0 + fd],
                                                 pt[:, :fd], Relu)
                        else:  # final group: drain on both engines so the
                            # closing store starts as early as possible
                            hh = fd // 2
                            nc.vector.tensor_scalar_max(
                                ot[:, o0:o0 + hh], pt[:, :hh], 0.0)
                            nc.scalar.activation(ot[:, o0 + hh:o0 + fd],
                                                 pt[:, hh:fd], Relu)
                        if g0 + fd >= stage_bounds[si + 1]:
                            nc.sync.dma_start(
                                out_d[kb * 128:(kb + 1) * 128, s0:g0 + fd],
                                ot[:, :g0 + fd - s0])
                            si += 1
                            ot = None
    return nc


def _legalize_multiwait(nc):
    """Split multi-wait instructions for this walrus build.

    The TPB instruction encodings carry exactly one semaphore wait, and
    the walrus codegen here refuses instructions with more ("Too many
    sync wait commands").  Hoist all but one wait onto EventSemaphore
    carrier instructions placed immediately before, on the same engine --
    the sequencer blocks on each carrier first, which is semantically
    identical.
    """
    import concourse.mybir as mybir

    for func in nc.m.functions:
        for blk in func.blocks:
            out = []
            changed = False
            for inst in blk.instructions:
                si = inst.sync_info
                waits = list(si.on_wait) if si is not None and si.on_wait else []
                if len(waits) > 1:
                    for j, w in enumerate(waits[:-1]):
                        carrier = mybir.InstEventSemaphore(
                            name=f"{inst.name}-xw{j}",
                            engine=inst.engine,
                            ins=[], outs=[],
                            sync_info=mybir.SyncInfo(on_wait=[w],
                                                     on_update=[]),
                        )
                        nc.register_instruction(carrier)
                        out.append(carrier)
                    inst.sync_info = mybir.SyncInfo(
                        on_wait=[waits[-1]],
                        on_update=list(si.on_update) if si.on_update else [])
                    changed = True
                out.append(inst)
            if changed:
                blk.instructions = out


_MODULES = {}


def _get_module(ncolf: int):
    if ncolf not in _MODULES:
        nc = _build_module(ncolf)
        _legalize_multiwait(nc)
        _MODULES[ncolf] = nc
    return _MODULES[ncolf]


def run(board_free, filters, areas, trace=False, **spmd_kwargs):
    from concourse.bass_utils import run_bass_kernel_spmd

    cols = _feasible_cols(filters)
    ncolf = len(cols)
    boardt = _build_boardt(board_free)
    mmat = _build_m(filters, areas, cols)

    in_maps = [
        {"boardt": boardt[c], "mmat": mmat}
        for c in range(N_CORES)
    ]
    nc = _get_module(ncolf)
    res = run_bass_kernel_spmd(nc, in_maps, core_ids=list(range(N_CORES)),
                               trace=trace, **spmd_kwargs)
    feas = np.concatenate(
        [np.asarray(r["out"]).astype(np.float32) for r in res.results], axis=0)
    out = np.zeros((BATCH, NCOL), dtype=np.float32)
    out[:, cols] = feas
    return out.reshape(BATCH, NF, 9, 9), res


def kernel(board_free, filters, areas):
    out, _ = run(board_free, filters, areas)
    return out


# revision 3
# speedup vs baseline: 2.2060x; 1.1370x over previous
"""Trainium2 Bass kernel for the deterministic legality module.

Computes, for each board b, filter f and top-left placement (i,j):
    legal[b,f,i,j] = 1.0 iff every occupied cell of filter f, placed at
    (i,j), lands in-bounds on a free cell of board b (and f is non-empty).

Two structural reductions over the dense formulation:

1. Feasibility pruning: a filter whose max occupied row is r and max
   occupied col is c can only be legal at the (9-r)*(9-c) top-left
   positions where its footprint stays in bounds -- every other (f,p)
   column of the output is constant zero (~68% of them).  Only feasible
   columns are computed on device; the host scatters them back.

2. Pair packing: two placements (same filter) share one matmul column
   with weights geo(p0) + 32*geo(p1) (entries {0,1,32,33}, exact in
   bf16).  The accumulator A = corr0 + 32*corr1 <= 825 is exact in f32
   and in the fp16 output; corr_i <= area_i <= 25 < 32 so the fields
   never interfere.  The host decodes corr_i = (A >> 5i) & 31 and
   compares with area_i.  This halves PE columns and, critically, the
   PSUM->SBUF drain (the PSUM read port of DVE+ACT is the pipeline
   bottleneck), at unchanged HBM store bytes (1 byte per placement).

Pipeline per core (batch 4096 -> 512 boards, 4 blocks of 128):
  bf16 matmul (K=81 padded to 128 partitions, N<=512) -> PSUM ring
  (4 slots of 1024 f32 cols) -> f32->fp16 copy drain split across
  DVE/ACT by greedy time balance ((120+FD)/0.96GHz vs (172+FD)/1.2GHz)
  -> SBUF staging -> HBM store on the SP HWDGE ring (1024-col stages
  so stores start early; the store wire time ~3.5MB is co-critical).
M slabs upload on the SP ring (tapered), boardT on the ACT ring.
Two short warmup matmuls keep the PE busy from the end of the
framework preamble so the HAM clock gate (~4.2us of continuous PE
activity at half clock) lifts as early as possible; dummy matmuls
after the real stream hold it lifted through the drain/store tail.
"""

import numpy as np
import ml_dtypes

N_CORES = 8
BATCH = 4096
BPC = BATCH // N_CORES  # 512 boards per core
NPOS = 81               # 9x9 board cells / placements
NF = 264                # filters
NCOL = NF * NPOS        # full output columns per board
KPAD = 128              # uploads padded to 128 partitions for DMA fan-out
PACK = 32               # field base: A = corr0 + 32*corr1

COL_TILE = 512          # one PSUM bank of f32
GRP = 1024              # PSUM ring slot / one drain op
STAGE = 1024            # output staging tile / store DMA granularity
_DVE_NS = lambda fd: (120.0 + fd) / 0.96
_ACT_NS = lambda fd: (172.0 + fd) / 1.2


def _plan_cols(filters: np.ndarray):
    """Pair feasible placements per filter.

    Returns (pairs, c0_idx, c1_idx, c1_valid): pairs = list of
    (col0, col1) full-output column ids with col1 == -1 for a dummy
    second half.
    """
    F = np.asarray(filters, dtype=np.float32).reshape(NF, 5, 5) > 0.5
    pairs = []
    for f in range(NF):
        occ = F[f]
        if not occ.any():
            continue
        rmax = int(np.where(occ.any(axis=1))[0].max())
        cmax = int(np.where(occ.any(axis=0))[0].max())
        cols = [f * NPOS + i * 9 + j
                for i in range(9 - rmax) for j in range(9 - cmax)]
        for k in range(0, len(cols) - 1, 2):
            pairs.append((cols[k], cols[k + 1]))
        if len(cols) % 2:
            pairs.append((cols[-1], -1))
    c0 = np.asarray([p[0] for p in pairs], dtype=np.int64)
    c1 = np.asarray([p[1] for p in pairs], dtype=np.int64)
    return pairs, c0, c1, c1 >= 0


def _geo(filters: np.ndarray) -> np.ndarray:
    """geo[81, 264*81] f32: filter f placed at position p, flattened."""
    F = np.asarray(filters, dtype=np.float32).reshape(NF, 5, 5)
    G = np.zeros((NPOS, NF, NPOS), dtype=np.float32)
    for i in range(9):
        h = min(5, 9 - i)
        for j in range(9):
            w = min(5, 9 - j)
            blk = np.zeros((NF, 9, 9), dtype=np.float32)
            blk[:, i:i + h, j:j + w] = F[:, :h, :w]
            G[:, :, i * 9 + j] = blk.reshape(NF, NPOS).T
    return G.reshape(NPOS, NF * NPOS)


def _build_m(filters: np.ndarray, c0: np.ndarray, c1: np.ndarray,
             c1v: np.ndarray) -> np.ndarray:
    """M [128, npair] bf16: geo(c0) + 32*geo(c1)."""
    G = _geo(filters)
    M = np.zeros((KPAD, len(c0)), dtype=np.float32)
    M[:NPOS] = G[:, c0]
    M[:NPOS, c1v] += PACK * G[:, c1[c1v]]
    return M.astype(ml_dtypes.bfloat16)


def _build_boardt(board_free: np.ndarray) -> np.ndarray:
    """boardT [cores, 128, 512] bf16: transposed boards, zero padded."""
    b = np.asarray(board_free, dtype=np.float32).reshape(N_CORES, BPC, NPOS)
    bt = np.zeros((N_CORES, KPAD, BPC), dtype=np.float32)
    bt[:, :NPOS, :] = b.transpose(0, 2, 1)
    return bt.astype(ml_dtypes.bfloat16)


def _drain_plan(npair: int):
    """Greedy DVE/ACT time-balanced [(block, col0, fd, engine)]."""
    plan = []
    tv = ts = 0.0
    for kb in range(4):
        for g0 in range(0, npair, GRP):
            fd = min(GRP, npair - g0)
            if kb == 3 and g0 + fd >= npair:
                plan.append((kb, g0, fd, 'split'))
            elif tv + _DVE_NS(fd) <= ts + _ACT_NS(fd):
                tv += _DVE_NS(fd)
                plan.append((kb, g0, fd, 'v'))
            else:
                ts += _ACT_NS(fd)
                plan.append((kb, g0, fd, 's'))
    return plan


def _build_module(npair: int):
    import concourse.bass as bass
    import concourse.mybir as mybir
    import concourse.tile as tile

    f32 = mybir.dt.float32
    f16 = mybir.dt.float16
    bf16 = mybir.dt.bfloat16
    fp8 = mybir.dt.float8e4

    nc = bass.Bass("TRN2", target_bir_lowering=False, debug=False,
                   num_devices=N_CORES)

    boardt_d = nc.dram_tensor("boardt", [KPAD, BPC], bf16,
                              kind="ExternalInput")
    m_d = nc.dram_tensor("mmat", [KPAD, npair], bf16, kind="ExternalInput")
    out_d = nc.dram_tensor("out", [BPC, npair], f16, kind="ExternalOutput")

    plan = _drain_plan(npair)

    with tile.TileContext(nc) as tc:
        with tc.tile_pool(name="const", bufs=1) as cpool:
            boardT = cpool.tile([KPAD, BPC], bf16)
            msb = cpool.tile([KPAD, npair], bf16)

            # M slabs on the SP hwdge ring with a taper (small leading
            # slab so the first matmuls start early); boardT on the ACT
            # ring so both uploads run in parallel.  Output stores
            # follow on the SP ring.
            bounds = [0, 512, 1536]
            while bounds[-1] < npair:
                bounds.append(min(npair, bounds[-1] + 1600))
            for s0, s1 in zip(bounds[:-1], bounds[1:]):
                nc.sync.dma_start(msb[:, s0:s1], m_d[:, s0:s1])
            nc.scalar.dma_start(boardT[:], boardt_d[:])

            with (
                tc.tile_pool(name="wprep", bufs=1) as wprep,
                tc.tile_pool(name="psM", bufs=4, space="PSUM") as psM,
                tc.tile_pool(name="ostage", bufs=6) as ostage,
            ):
                # short warm-up matmuls on memset zeros: PE-busy starts
                # at the end of the framework preamble instead of when
                # the first M slab lands, pulling the HAM gate lift
                # earlier without delaying the first real matmul.
                wz = wprep.tile([128, 256], bf16, tag="wz")
                nc.vector.memset(wz[:], 0.0)
                wps = psM.tile([128, GRP], f32, tag="mm")
                for _ in range(2):
                    nc.tensor.matmul(wps[:, 0:256], wz[:, 0:128], wz[:],
                                     start=True, stop=True)

                nkb = BPC // 128
                tails = []
                for kb in range(nkb):
                    lhsT = boardT[:, kb * 128:(kb + 1) * 128]
                    ot = None
                    for (pkb, g0, fd, eng) in plan:
                        if pkb != kb:
                            continue
                        if ot is None:
                            s0 = g0
                            ot = ostage.tile([128, STAGE], f16, tag="ot")
                        pt = psM.tile([128, GRP], f32, tag="mm")
                        for q in range(0, fd, COL_TILE):
                            w = min(COL_TILE, fd - q)
                            nc.tensor.matmul(pt[:, q:q + w], lhsT,
                                             msb[:, g0 + q:g0 + q + w],
                                             start=True, stop=True)
                        if kb == nkb - 1:
                            tails.append(pt)
                        o0 = g0 - s0
                        if eng == 'v':
                            nc.vector.tensor_scalar_max(
                                ot[:, o0:o0 + fd], pt[:, :fd], 0.0)
                        elif eng == 's':
                            nc.scalar.activation(
                                ot[:, o0:o0 + fd], pt[:, :fd],
                                mybir.ActivationFunctionType.Copy)
                        else:  # final group: drain on both engines so
                            # the closing store starts as early as possible
                            hh = fd // 2
                            nc.vector.tensor_scalar_max(
                                ot[:, o0:o0 + hh], pt[:, :hh], 0.0)
                            nc.scalar.activation(
                                ot[:, o0 + hh:o0 + fd], pt[:, hh:fd],
                                mybir.ActivationFunctionType.Copy)
                        if g0 + fd - s0 >= STAGE or g0 + fd >= npair:
                            nc.sync.dma_start(
                                out_d[kb * 128:(kb + 1) * 128,
                                      s0:g0 + fd],
                                ot[:, :g0 + fd - s0])
                            ot = None
                # dummy matmuls into already-drained tail slots: keep
                # the PE busy so the HAM clock gate stays lifted while
                # the last drains and stores run.
                for pt in tails[-2:]:
                    for _ in range(3):
                        nc.tensor.matmul(pt[:, 0:256], wz[:, 0:128],
                                         wz[:], start=True, stop=True)
    return nc


def _legalize_multiwait(nc):
    """Split multi-wait instructions for this walrus build.

    The TPB instruction encodings carry exactly one semaphore wait, and
    the walrus codegen here refuses instructions with more ("Too many
    sync wait commands").  Hoist all but one wait onto EventSemaphore
    carrier instructions placed immediately before, on the same engine --
    the sequencer blocks on each carrier first, which is semantically
    identical.
    """
    import concourse.mybir as mybir

    for func in nc.m.functions:
        for blk in func.blocks:
            out = []
            changed = False
            for inst in blk.instructions:
                si = inst.sync_info
                waits = list(si.on_wait) if si is not None and si.on_wait else []
                if len(waits) > 1:
                    for j, w in enumerate(waits[:-1]):
                        carrier = mybir.InstEventSemaphore(
                            name=f"{inst.name}-xw{j}",
                            engine=inst.engine,
                            ins=[], outs=[],
                            sync_info=mybir.SyncInfo(on_wait=[w],
                                                     on_update=[]),
                        )
                        nc.register_instruction(carrier)
                        out.append(carrier)
                    inst.sync_info = mybir.SyncInfo(
                        on_wait=[waits[-1]],
                        on_update=list(si.on_update) if si.on_update else [])
                    changed = True
                out.append(inst)
            if changed:
                blk.instructions = out


_MODULES = {}


def _get_module(npair: int):
    if npair not in _MODULES:
        nc = _build_module(npair)
        _legalize_multiwait(nc)
        _MODULES[npair] = nc
    return _MODULES[npair]


def run(board_free, filters, areas, trace=False, **spmd_kwargs):
    from concourse.bass_utils import run_bass_kernel_spmd

    pairs, c0, c1, c1v = _plan_cols(filters)
    npair = len(pairs)
    boardt = _build_boardt(board_free)
    mmat = _build_m(filters, c0, c1, c1v)

    in_maps = [
        {"boardt": boardt[c], "mmat": mmat}
        for c in range(N_CORES)
    ]
    nc = _get_module(npair)
    res = run_bass_kernel_spmd(nc, in_maps, core_ids=list(range(N_CORES)),
                               trace=trace, **spmd_kwargs)
    A = np.concatenate(
        [np.asarray(r["out"]) for r in res.results],
        axis=0).astype(np.int32)  # exact integers <= 825

    ar = np.asarray(areas, dtype=np.int32).reshape(NF)
    a0 = ar[c0 // NPOS]
    corr0 = A & (PACK - 1)
    corr1 = A >> 5
    out = np.zeros((BATCH, NCOL), dtype=np.float32)
    out[:, c0] = (corr0 == a0[None, :]).astype(np.float32)
    a1 = ar[c1[c1v] // NPOS]
    out[:, c1[c1v]] = (corr1[:, c1v] == a1[None, :]).astype(np.float32)
    return out.reshape(BATCH, NF, 9, 9), res


def kernel(board_free, filters, areas):
    out, _ = run(board_free, filters, areas)
    return out
